# revision 25
# baseline (speedup 1.0000x reference)
"""Trainium2 Bass kernel for nn_MultiHeadAttention_84576495993495.

Key observation: the reference module's output einsum is
    out = einsum('bhqk,bhvo->bhvo', attn, v)
which contracts softmax(attn) over BOTH q and k. Every softmax row sums
to 1, so sum_{q,k} attn == S (= 2048) and the whole attention block
collapses to out == S * v. Hence

    reference(x, ...) == ((x @ Wv.T + bv) * S) @ Wp.T + bp
                      ==  x @ M + c
with
    M = S * Wv.T @ Wp.T          (folded on host in fp64, then split)
    c = S * Wp @ bv + bp

(Verified vs the jax reference: rel Frobenius err ~3.6e-7 = fp32 noise.)

Device work: the data-dependent GEMM y = x @ M + c, sharded
data-parallel over the 8192 rows -> 1024 rows per NeuronCore.

Precision strategy: TensorE native fp32 matmul runs at 4 cyc/row (and
measures ~2x worse than that on HW); fp16 runs at 1 cyc/row.  So x and
M are each split into a high + low fp16 pair (x = xh + xl, M = Mh + Ml,
each pair exact to ~2^-22 relative) and the GEMM is computed as three
fp16 passes accumulated in the same fp32 PSUM group:
    y = xh@Mh + xh@Ml + xl@Mh  (+ c)
The dropped xl@Ml term is ~2^-22 relative -- below fp32 round-off for
this problem.  CPU-verified: rel err 3.56e-7, identical to a pure-fp32
evaluation of the same GEMM.

Layout: the TensorE contracts over the partition dim, so the x shard is
fed pre-transposed (host-side layout prep; fp32/fp16 DMA-transpose of
the activation on-device is not worth it here).  Per n-chunk half, the
schedule is k-major across 8 live PSUM banks so the PE only ever waits
for one (x[k], M[k]) tile pair (~384 KB) instead of the whole working
set, and the moving operand (rhs) stays constant across each 8-matmul
inner sweep (measured faster than chaining each bank's accumulation
contiguously: 137.7 vs 160.5 us steady-state).

Measured on HW (8 cores, axon): rel err vs reference 2.554e-07
(absmax 2.2e-3 on a 5.3e+3 scale); steady-state body time ~138 us/core
(For_i loop slope over T in {1, 8193, 16385}); native-fp32 variant of
the same kernel measures ~247 us, float32r ~matches fp16x3 speed but
with rel err 1.25e-4.
"""

import os
from functools import lru_cache

import numpy as np

# Defensive: a previous run crashing mid-execution can leave the NeuronCores
# in an unrecoverable state (NRT_EXEC_UNIT_UNRECOVERABLE); resetting cores at
# NRT init clears it and is harmless otherwise.
os.environ.setdefault("NEURON_RT_RESET_CORES", "1")

import concourse.bass as bass
import concourse.mybir as mybir
import concourse.tile as tile
from concourse import bacc
from concourse.bass_utils import run_bass_kernel_spmd

N_CORES = 8
P = 128
D = 1024                       # model dim (= SLICE_SIZE)
B, S = 4, 2048
R_TOTAL = B * S                # 8192 rows
R_CORE = R_TOTAL // N_CORES    # 1024 rows per core
K_TILES = D // P               # 8
R_TILES = R_CORE // P          # 8
N_CHUNK = 512                  # one PSUM bank / fp32 moving-operand max
N_CHUNKS = D // N_CHUNK        # 2
SCALE = float(S)               # sum over q,k of softmax rows == S

# "fp16x1" (default) | "fp16x3" | "float32" | "float32r"
# fp16x1: single fp16 pass (xh@Mh only). CPU-verified rel err 2.50e-4 vs
# the reference -- 80x under the 2e-2 gate -- at 1/3 the TensorE work of
# fp16x3 (128 vs 384 matmul instructions per core).
MM_MODE = os.environ.get("KMM_DTYPE", "fp16x1")
MM_ORDER = os.environ.get("KMM_ORDER", "kmajor")
# Loop-unroll factor for the For_i steady-state benchmark NEFFs: tc.For_i
# runs an InstAllEngineBarrier + semaphore-reset block between iterations,
# so UNROLL bodies are emitted per iteration to amortize it. The reported
# per-body time is slope/UNROLL.
UNROLL = int(os.environ.get("KMM_UNROLL", "4"))


@lru_cache(maxsize=8)
def _build_nc(
    mode: str,
    loop_iters: int | None = None,
    order: str | None = None,
    unroll: int | None = None,
):
    """loop_iters: when set, wrap the compute body in a tc.For_i hardware
    loop (inputs loaded once) -- used by the benchmark harness to measure
    steady-state per-iteration device time without NTFF profiling."""
    if order is None:
        order = MM_ORDER
    if unroll is None:
        unroll = UNROLL if loop_iters is not None else 1
    split = mode == "fp16x3"
    mm_dt = mybir.dt.float16 if mode.startswith("fp16") else getattr(mybir.dt, mode)
    nc = bacc.Bacc(None, target_bir_lowering=False)

    if split:
        x_names, m_names = ["xh", "xl"], ["Mh", "Ml"]
    else:
        x_names, m_names = ["xh"], ["Mh"]
    mstat = order == "mstat"
    # "lean" production path: no on-device bias (host adds it after the
    # gather), fp16 output (halves drain + output-DMA cost; adds ~2.4e-4
    # fp16 rounding, well under the 2e-2 gate), drains alternating between
    # the Scalar and Vector engines.
    lean = order == "kmajor"
    x_dram = [
        nc.dram_tensor(n, [D, R_CORE], mm_dt, kind="ExternalInput") for n in x_names
    ]
    m_dram = [nc.dram_tensor(n, [D, D], mm_dt, kind="ExternalInput") for n in m_names]
    cb = None
    if not lean:
        # mstat: bias laid out [P, n_tile] (per-partition scalars); output y^T.
        cb = nc.dram_tensor(
            "cb",
            [P, K_TILES] if mstat else [P, D],
            mybir.dt.float32,
            kind="ExternalInput",
        )
    out_dt = mybir.dt.float16 if lean else mybir.dt.float32
    y = nc.dram_tensor(
        "y",
        [D, R_CORE] if mstat else [R_CORE, D],
        out_dt,
        kind="ExternalOutput",
    )

    x_t = [t.rearrange("(ko p) r -> p ko r", p=P) for t in x_dram]   # [128, 8, 1024]
    m_t = [t.rearrange("(ko p) n -> p ko n", p=P) for t in m_dram]   # [128, 8, 1024]

    # (x operand, M operand) per accumulation pass; the xl@Ml term is dropped.
    passes = [(0, 0), (0, 1), (1, 0)] if split else [(0, 0)]

    with tile.TileContext(nc) as tc:
        with (
            tc.tile_pool(name="wpool", bufs=1) as wpool,
            tc.tile_pool(name="opool", bufs=8) as opool,
            tc.tile_pool(name="pspool", bufs=8, space="PSUM") as pspool,
        ):
            x_sb = [
                wpool.tile([P, K_TILES, R_CORE], mm_dt, tag=f"x_sb{i}", name=f"x_sb{i}")
                for i in range(len(x_dram))
            ]
            m_sb = [
                wpool.tile([P, K_TILES, D], mm_dt, tag=f"m_sb{i}", name=f"m_sb{i}")
                for i in range(len(m_dram))
            ]
            cb_sb = None
            if not lean:
                cb_sb = wpool.tile(
                    [P, K_TILES] if mstat else [P, D], mybir.dt.float32, tag="cb_sb"
                )
                nc.sync.dma_start(cb_sb[:], cb[:])
            # Load in pass-0 consumption order first (xh, Mh), then the
            # low halves; per-k granularity so the PE can chase the stream.
            for i in range(len(x_dram)):
                for k in range(K_TILES):
                    nc.sync.dma_start(x_sb[i][:, k], x_t[i][:, k])
                    for nch in range(N_CHUNKS):
                        nc.sync.dma_start(
                            m_sb[i][:, k, bass.ts(nch, N_CHUNK)],
                            m_t[i][:, k, bass.ts(nch, N_CHUNK)],
                        )

            n_acc = len(passes) * K_TILES

            def emit_tail(r, nch, ps):
                if lean:
                    # Pure PSUM->SBUF fp16 drain (bias added on host);
                    # alternate Scalar/Vector so the 8-drain burst at each
                    # chunk boundary halves in duration and the first bank
                    # the next chunk needs is recycled sooner.
                    out_sb = opool.tile([P, N_CHUNK], out_dt, tag="out_sb")
                    if r % 2 == 0:
                        nc.scalar.copy(out_sb[:], ps[:])
                    else:
                        nc.vector.tensor_scalar_add(out_sb[:], ps[:], 0.0)
                else:
                    out_sb = opool.tile([P, N_CHUNK], mybir.dt.float32, tag="out_sb")
                    nc.vector.tensor_add(
                        out_sb[:], ps[:], cb_sb[:, bass.ts(nch, N_CHUNK)]
                    )
                nc.sync.dma_start(
                    y[bass.ts(r, P), bass.ts(nch, N_CHUNK)], out_sb[:]
                )

            def body_kmajor():
                # k-major across 8 live PSUM banks (bank switch every MM)
                for nch in range(N_CHUNKS):
                    groups = [
                        pspool.tile([P, N_CHUNK], mybir.dt.float32, tag="ps", name="ps")
                        for _ in range(R_TILES)
                    ]
                    step = 0
                    for xi, mi in passes:
                        for k in range(K_TILES):
                            for r in range(R_TILES):
                                nc.tensor.matmul(
                                    groups[r][:],
                                    x_sb[xi][:, k, bass.ts(r, P)],
                                    m_sb[mi][:, k, bass.ts(nch, N_CHUNK)],
                                    start=(step == 0),
                                    stop=(step == n_acc - 1),
                                )
                            step += 1
                    for r in range(R_TILES):
                        emit_tail(r, nch, groups[r])

            def body_mstat():
                # M-stationary: per (n, k) the weight tile M[k, n] is loaded
                # once and both x row-chunks stream through it, so half the
                # matmuls reuse the already-loaded stationary operand.
                # Output comes out transposed (y^T tiles [128 cols, 512 rows]);
                # the host transposes back. Bias becomes a per-partition
                # scalar add fused into the PSUM drain. Drains (2 per n-tile)
                # are spread evenly instead of bursting at a chunk boundary.
                for n in range(K_TILES):
                    tiles = [
                        pspool.tile([P, N_CHUNK], mybir.dt.float32, tag="ps", name="ps")
                        for _ in range(N_CHUNKS)
                    ]
                    for xi, mi in passes:
                        for k in range(K_TILES):
                            for rc in range(N_CHUNKS):
                                nc.tensor.matmul(
                                    tiles[rc][:],
                                    m_sb[mi][:, k, bass.ts(n, P)],
                                    x_sb[xi][:, k, bass.ts(rc, N_CHUNK)],
                                    start=((xi, mi) == passes[0] and k == 0),
                                    stop=(
                                        (xi, mi) == passes[-1] and k == K_TILES - 1
                                    ),
                                )
                    for rc in range(N_CHUNKS):
                        out_sb = opool.tile([P, N_CHUNK], mybir.dt.float32, tag="out_sb")
                        nc.vector.tensor_scalar_add(
                            out_sb[:], tiles[rc][:], cb_sb[:, n]
                        )
                        nc.sync.dma_start(
                            y[bass.ts(n, P), bass.ts(rc, N_CHUNK)], out_sb[:]
                        )

            def body_probe_same():
                # Diagnostic only (wrong output): identical operands for every
                # MM. If this still runs at ~kmajor speed, the per-MM overhead
                # is stream-start latency, not the weight reload.
                for nch in range(N_CHUNKS):
                    groups = [
                        pspool.tile([P, N_CHUNK], mybir.dt.float32, tag="ps", name="ps")
                        for _ in range(R_TILES)
                    ]
                    step = 0
                    for _ in passes:
                        for k in range(K_TILES):
                            for r in range(R_TILES):
                                nc.tensor.matmul(
                                    groups[r][:],
                                    x_sb[0][:, 0, bass.ts(0, P)],
                                    m_sb[0][:, 0, bass.ts(0, N_CHUNK)],
                                    start=(step == 0),
                                    stop=(step == n_acc - 1),
                                )
                            step += 1
                    for r in range(R_TILES):
                        emit_tail(r, nch, groups[r])

            def body_reuse2():
                # Two halves of 4 r-tiles; per half, 8 live PSUM banks =
                # 4 r-tiles x 2 n-chunks. Inner pair shares the stationary
                # x[k,r] across both n-chunks (2 MMs per weight load), and
                # each half's drains overlap the other half's matmuls, so
                # the PE never waits on a PSUM WAR at a chunk boundary.
                for half in range(2):
                    rs = [4 * half + j for j in range(4)]
                    groups = {
                        (r, nch): pspool.tile(
                            [P, N_CHUNK], mybir.dt.float32, tag="ps", name="ps"
                        )
                        for r in rs
                        for nch in range(N_CHUNKS)
                    }
                    for xi, mi in passes:
                        for k in range(K_TILES):
                            for r in rs:
                                for nch in range(N_CHUNKS):
                                    step_first = (xi, mi) == passes[0] and k == 0
                                    step_last = (xi, mi) == passes[-1] and k == (
                                        K_TILES - 1
                                    )
                                    nc.tensor.matmul(
                                        groups[(r, nch)][:],
                                        x_sb[xi][:, k, bass.ts(r, P)],
                                        m_sb[mi][:, k, bass.ts(nch, N_CHUNK)],
                                        start=step_first,
                                        stop=step_last,
                                    )
                    for r in rs:
                        for nch in range(N_CHUNKS):
                            emit_tail(r, nch, groups[(r, nch)])

            def body_chain():
                # group-major: each PSUM bank's accumulation chain runs as
                # consecutive MMs (no bank cycling between accumulate steps)
                for nch in range(N_CHUNKS):
                    for r in range(R_TILES):
                        ps = pspool.tile(
                            [P, N_CHUNK], mybir.dt.float32, tag="ps", name="ps"
                        )
                        step = 0
                        for xi, mi in passes:
                            for k in range(K_TILES):
                                nc.tensor.matmul(
                                    ps[:],
                                    x_sb[xi][:, k, bass.ts(r, P)],
                                    m_sb[mi][:, k, bass.ts(nch, N_CHUNK)],
                                    start=(step == 0),
                                    stop=(step == n_acc - 1),
                                )
                                step += 1
                        emit_tail(r, nch, ps)

            body = {
                "chain": body_chain,
                "kmajor": body_kmajor,
                "reuse2": body_reuse2,
                "mstat": body_mstat,
                "probe_same": body_probe_same,
            }[order]

            if loop_iters is None:
                body()
            else:
                with tc.For_i(0, loop_iters, 1):
                    for _ in range(unroll):
                        body()
    nc.compile()
    return nc


def _fold_c(Wv, bv, Wp, bp):
    return (SCALE * (Wp.astype(np.float64) @ bv.astype(np.float64)) + bp).astype(
        np.float32
    )


def _assemble(res, c, order=None):
    """Gather per-core outputs into the full [R_TOTAL, D] fp32 GEMM result."""
    order = order or MM_ORDER
    shards = [r["y"].T if order == "mstat" else r["y"] for r in res.results]
    y = np.concatenate(shards, axis=0).astype(np.float32)
    if order == "kmajor":
        y = y + c  # bias is not applied on device in the lean path
    return y


def _host_prep(x, Wv, bv, Wp, bp, mode=None, order=None):
    mode = mode or MM_MODE
    order = order or MM_ORDER
    X = np.ascontiguousarray(x, dtype=np.float32).reshape(R_TOTAL, D)
    M64 = SCALE * (Wv.T.astype(np.float64) @ Wp.T.astype(np.float64))
    c = _fold_c(Wv, bv, Wp, bp)
    if order == "mstat":
        # Per-partition bias: column block n of y^T gets c[n*128:(n+1)*128].
        cbt = np.ascontiguousarray(c.reshape(K_TILES, P).T)
    else:
        cbt = np.ascontiguousarray(np.broadcast_to(c, (P, D)))

    if mode == "fp16x3":
        Mh = M64.astype(np.float16)
        Ml = (M64 - Mh.astype(np.float64)).astype(np.float16)
        m_arrs = {"Mh": Mh, "Ml": Ml}
    elif mode == "fp16x1":
        m_arrs = {"Mh": M64.astype(np.float16)}
    else:
        m_arrs = {"Mh": M64.astype(np.float32)}

    in_maps = []
    for i in range(N_CORES):
        shard_t = np.ascontiguousarray(X[i * R_CORE : (i + 1) * R_CORE].T)
        im = dict(m_arrs)
        if order != "kmajor":
            im["cb"] = cbt
        if mode == "fp16x3":
            xh = shard_t.astype(np.float16)
            xl = (shard_t - xh.astype(np.float32)).astype(np.float16)
            im["xh"] = xh
            im["xl"] = xl
        elif mode == "fp16x1":
            im["xh"] = shard_t.astype(np.float16)
        else:
            im["xh"] = shard_t
        in_maps.append(im)
    return in_maps


def kernel(x, Wq, bq, Wk, bk, Wv, bv, Wp, bp):
    x, Wv, bv, Wp, bp = (np.asarray(a) for a in (x, Wv, bv, Wp, bp))
    nc = _build_nc(MM_MODE)
    in_maps = _host_prep(x, Wv, bv, Wp, bp)
    res = run_bass_kernel_spmd(nc, in_maps, core_ids=list(range(N_CORES)))
    y = _assemble(res, _fold_c(Wv, bv, Wp, bp))
    return np.ascontiguousarray(y).reshape(B, S, D)



# revision 26
# speedup vs baseline: 1.0353x; 1.0353x over previous
"""Trainium2 Bass kernel for nn_MultiHeadAttention_84576495993495.

Key observation: the reference module's output einsum is
    out = einsum('bhqk,bhvo->bhvo', attn, v)
which contracts softmax(attn) over BOTH q and k. Every softmax row sums
to 1, so sum_{q,k} attn == S (= 2048) and the whole attention block
collapses to out == S * v. Hence

    reference(x, ...) == ((x @ Wv.T + bv) * S) @ Wp.T + bp
                      ==  x @ M + c
with
    M = S * Wv.T @ Wp.T          (folded on host in fp64, then split)
    c = S * Wp @ bv + bp

(Verified vs the jax reference: rel Frobenius err ~3.6e-7 = fp32 noise.)

Device work: the data-dependent GEMM y = x @ M + c, sharded
data-parallel over the 8192 rows -> 1024 rows per NeuronCore.

Precision strategy: TensorE native fp32 matmul runs at 4 cyc/row (and
measures ~2x worse than that on HW); fp16 runs at 1 cyc/row.  So x and
M are each split into a high + low fp16 pair (x = xh + xl, M = Mh + Ml,
each pair exact to ~2^-22 relative) and the GEMM is computed as three
fp16 passes accumulated in the same fp32 PSUM group:
    y = xh@Mh + xh@Ml + xl@Mh  (+ c)
The dropped xl@Ml term is ~2^-22 relative -- below fp32 round-off for
this problem.  CPU-verified: rel err 3.56e-7, identical to a pure-fp32
evaluation of the same GEMM.

Layout: the TensorE contracts over the partition dim, so the x shard is
fed pre-transposed (host-side layout prep; fp32/fp16 DMA-transpose of
the activation on-device is not worth it here).  Per n-chunk half, the
schedule is k-major across 8 live PSUM banks so the PE only ever waits
for one (x[k], M[k]) tile pair (~384 KB) instead of the whole working
set, and the moving operand (rhs) stays constant across each 8-matmul
inner sweep (measured faster than chaining each bank's accumulation
contiguously: 137.7 vs 160.5 us steady-state).

Measured on HW (8 cores, axon): rel err vs reference 2.554e-07
(absmax 2.2e-3 on a 5.3e+3 scale); steady-state body time ~138 us/core
(For_i loop slope over T in {1, 8193, 16385}); native-fp32 variant of
the same kernel measures ~247 us, float32r ~matches fp16x3 speed but
with rel err 1.25e-4.
"""

import os
from functools import lru_cache

import numpy as np

# Defensive: a previous run crashing mid-execution can leave the NeuronCores
# in an unrecoverable state (NRT_EXEC_UNIT_UNRECOVERABLE); resetting cores at
# NRT init clears it and is harmless otherwise.
os.environ.setdefault("NEURON_RT_RESET_CORES", "1")

import concourse.bass as bass
import concourse.mybir as mybir
import concourse.tile as tile
from concourse import bacc
from concourse.bass_utils import run_bass_kernel_spmd

N_CORES = 8
P = 128
D = 1024                       # model dim (= SLICE_SIZE)
B, S = 4, 2048
R_TOTAL = B * S                # 8192 rows
R_CORE = R_TOTAL // N_CORES    # 1024 rows per core
K_TILES = D // P               # 8
R_TILES = R_CORE // P          # 8
N_CHUNK = 512                  # one PSUM bank / fp32 moving-operand max
N_CHUNKS = D // N_CHUNK        # 2
SCALE = float(S)               # sum over q,k of softmax rows == S

# "fp16x1" (default) | "fp16x3" | "float32" | "float32r"
# fp16x1: single fp16 pass (xh@Mh only). CPU-verified rel err 2.50e-4 vs
# the reference -- 80x under the 2e-2 gate -- at 1/3 the TensorE work of
# fp16x3 (128 vs 384 matmul instructions per core).
MM_MODE = os.environ.get("KMM_DTYPE", "fp16x1")
MM_ORDER = os.environ.get("KMM_ORDER", "kmajor")
# Loop-unroll factor for the For_i steady-state benchmark NEFFs: tc.For_i
# runs an InstAllEngineBarrier + semaphore-reset block between iterations,
# so UNROLL bodies are emitted per iteration to amortize it. The reported
# per-body time is slope/UNROLL.
UNROLL = int(os.environ.get("KMM_UNROLL", "4"))


@lru_cache(maxsize=8)
def _build_nc(
    mode: str,
    loop_iters: int | None = None,
    order: str | None = None,
    unroll: int | None = None,
):
    """loop_iters: when set, wrap the compute body in a tc.For_i hardware
    loop (inputs loaded once) -- used by the benchmark harness to measure
    steady-state per-iteration device time without NTFF profiling."""
    if order is None:
        order = MM_ORDER
    if unroll is None:
        unroll = UNROLL if loop_iters is not None else 1
    split = mode == "fp16x3"
    mm_dt = mybir.dt.float16 if mode.startswith("fp16") else getattr(mybir.dt, mode)
    nc = bacc.Bacc(None, target_bir_lowering=False)

    if split:
        x_names, m_names = ["xh", "xl"], ["Mh", "Ml"]
    else:
        x_names, m_names = ["xh"], ["Mh"]
    mstat = order == "mstat"
    # "lean" production path: no on-device bias (host adds it after the
    # gather), fp16 output (halves drain + output-DMA cost; adds ~2.4e-4
    # fp16 rounding, well under the 2e-2 gate), drains alternating between
    # the Scalar and Vector engines.
    lean = order == "kmajor"
    x_dram = [
        nc.dram_tensor(n, [D, R_CORE], mm_dt, kind="ExternalInput") for n in x_names
    ]
    m_dram = [nc.dram_tensor(n, [D, D], mm_dt, kind="ExternalInput") for n in m_names]
    cb = None
    if not lean:
        # mstat: bias laid out [P, n_tile] (per-partition scalars); output y^T.
        cb = nc.dram_tensor(
            "cb",
            [P, K_TILES] if mstat else [P, D],
            mybir.dt.float32,
            kind="ExternalInput",
        )
    out_dt = mybir.dt.float16 if lean else mybir.dt.float32
    y = nc.dram_tensor(
        "y",
        [D, R_CORE] if mstat else [R_CORE, D],
        out_dt,
        kind="ExternalOutput",
    )

    x_t = [t.rearrange("(ko p) r -> p ko r", p=P) for t in x_dram]   # [128, 8, 1024]
    m_t = [t.rearrange("(ko p) n -> p ko n", p=P) for t in m_dram]   # [128, 8, 1024]

    # (x operand, M operand) per accumulation pass; the xl@Ml term is dropped.
    passes = [(0, 0), (0, 1), (1, 0)] if split else [(0, 0)]

    with tile.TileContext(nc) as tc:
        with (
            tc.tile_pool(name="wpool", bufs=1) as wpool,
            tc.tile_pool(name="opool", bufs=8) as opool,
            tc.tile_pool(name="pspool", bufs=8, space="PSUM") as pspool,
        ):
            x_sb = [
                wpool.tile([P, K_TILES, R_CORE], mm_dt, tag=f"x_sb{i}", name=f"x_sb{i}")
                for i in range(len(x_dram))
            ]
            m_sb = [
                wpool.tile([P, K_TILES, D], mm_dt, tag=f"m_sb{i}", name=f"m_sb{i}")
                for i in range(len(m_dram))
            ]
            cb_sb = None
            if not lean:
                cb_sb = wpool.tile(
                    [P, K_TILES] if mstat else [P, D], mybir.dt.float32, tag="cb_sb"
                )
                nc.sync.dma_start(cb_sb[:], cb[:])
            # Load in pass-0 consumption order first (xh, Mh), then the
            # low halves; per-k granularity so the PE can chase the stream.
            for i in range(len(x_dram)):
                for k in range(K_TILES):
                    nc.sync.dma_start(x_sb[i][:, k], x_t[i][:, k])
                    for nch in range(N_CHUNKS):
                        nc.sync.dma_start(
                            m_sb[i][:, k, bass.ts(nch, N_CHUNK)],
                            m_t[i][:, k, bass.ts(nch, N_CHUNK)],
                        )

            n_acc = len(passes) * K_TILES

            def emit_tail(r, nch, ps):
                if lean:
                    # Pure PSUM->SBUF fp16 drain (bias added on host);
                    # alternate Scalar/Vector so the 8-drain burst at each
                    # chunk boundary halves in duration and the first bank
                    # the next chunk needs is recycled sooner.
                    out_sb = opool.tile([P, N_CHUNK], out_dt, tag="out_sb")
                    if r % 2 == 0:
                        nc.scalar.copy(out_sb[:], ps[:])
                    else:
                        nc.vector.tensor_scalar_add(out_sb[:], ps[:], 0.0)
                else:
                    out_sb = opool.tile([P, N_CHUNK], mybir.dt.float32, tag="out_sb")
                    nc.vector.tensor_add(
                        out_sb[:], ps[:], cb_sb[:, bass.ts(nch, N_CHUNK)]
                    )
                nc.sync.dma_start(
                    y[bass.ts(r, P), bass.ts(nch, N_CHUNK)], out_sb[:]
                )

            def body_kmajor():
                # k-major across 8 live PSUM banks (bank switch every MM)
                for nch in range(N_CHUNKS):
                    groups = [
                        pspool.tile([P, N_CHUNK], mybir.dt.float32, tag="ps", name="ps")
                        for _ in range(R_TILES)
                    ]
                    step = 0
                    for xi, mi in passes:
                        for k in range(K_TILES):
                            for r in range(R_TILES):
                                nc.tensor.matmul(
                                    groups[r][:],
                                    x_sb[xi][:, k, bass.ts(r, P)],
                                    m_sb[mi][:, k, bass.ts(nch, N_CHUNK)],
                                    start=(step == 0),
                                    stop=(step == n_acc - 1),
                                )
                            step += 1
                    for r in range(R_TILES):
                        emit_tail(r, nch, groups[r])

            def body_mstat():
                # M-stationary: per (n, k) the weight tile M[k, n] is loaded
                # once and both x row-chunks stream through it, so half the
                # matmuls reuse the already-loaded stationary operand.
                # Output comes out transposed (y^T tiles [128 cols, 512 rows]);
                # the host transposes back. Bias becomes a per-partition
                # scalar add fused into the PSUM drain. Drains (2 per n-tile)
                # are spread evenly instead of bursting at a chunk boundary.
                for n in range(K_TILES):
                    tiles = [
                        pspool.tile([P, N_CHUNK], mybir.dt.float32, tag="ps", name="ps")
                        for _ in range(N_CHUNKS)
                    ]
                    for xi, mi in passes:
                        for k in range(K_TILES):
                            for rc in range(N_CHUNKS):
                                nc.tensor.matmul(
                                    tiles[rc][:],
                                    m_sb[mi][:, k, bass.ts(n, P)],
                                    x_sb[xi][:, k, bass.ts(rc, N_CHUNK)],
                                    start=((xi, mi) == passes[0] and k == 0),
                                    stop=(
                                        (xi, mi) == passes[-1] and k == K_TILES - 1
                                    ),
                                )
                    for rc in range(N_CHUNKS):
                        out_sb = opool.tile([P, N_CHUNK], mybir.dt.float32, tag="out_sb")
                        nc.vector.tensor_scalar_add(
                            out_sb[:], tiles[rc][:], cb_sb[:, n]
                        )
                        nc.sync.dma_start(
                            y[bass.ts(n, P), bass.ts(rc, N_CHUNK)], out_sb[:]
                        )

            def body_probe_same():
                # Diagnostic only (wrong output): identical operands for every
                # MM. If this still runs at ~kmajor speed, the per-MM overhead
                # is stream-start latency, not the weight reload.
                for nch in range(N_CHUNKS):
                    groups = [
                        pspool.tile([P, N_CHUNK], mybir.dt.float32, tag="ps", name="ps")
                        for _ in range(R_TILES)
                    ]
                    step = 0
                    for _ in passes:
                        for k in range(K_TILES):
                            for r in range(R_TILES):
                                nc.tensor.matmul(
                                    groups[r][:],
                                    x_sb[0][:, 0, bass.ts(0, P)],
                                    m_sb[0][:, 0, bass.ts(0, N_CHUNK)],
                                    start=(step == 0),
                                    stop=(step == n_acc - 1),
                                )
                            step += 1
                    for r in range(R_TILES):
                        emit_tail(r, nch, groups[r])

            def body_reuse2():
                # Two halves of 4 r-tiles; per half, 8 live PSUM banks =
                # 4 r-tiles x 2 n-chunks. Inner pair shares the stationary
                # x[k,r] across both n-chunks (2 MMs per weight load), and
                # each half's drains overlap the other half's matmuls, so
                # the PE never waits on a PSUM WAR at a chunk boundary.
                for half in range(2):
                    rs = [4 * half + j for j in range(4)]
                    groups = {
                        (r, nch): pspool.tile(
                            [P, N_CHUNK], mybir.dt.float32, tag="ps", name="ps"
                        )
                        for r in rs
                        for nch in range(N_CHUNKS)
                    }
                    for xi, mi in passes:
                        for k in range(K_TILES):
                            for r in rs:
                                for nch in range(N_CHUNKS):
                                    step_first = (xi, mi) == passes[0] and k == 0
                                    step_last = (xi, mi) == passes[-1] and k == (
                                        K_TILES - 1
                                    )
                                    nc.tensor.matmul(
                                        groups[(r, nch)][:],
                                        x_sb[xi][:, k, bass.ts(r, P)],
                                        m_sb[mi][:, k, bass.ts(nch, N_CHUNK)],
                                        start=step_first,
                                        stop=step_last,
                                    )
                    for r in rs:
                        for nch in range(N_CHUNKS):
                            emit_tail(r, nch, groups[(r, nch)])

            def body_chain():
                # group-major: each PSUM bank's accumulation chain runs as
                # consecutive MMs (no bank cycling between accumulate steps)
                for nch in range(N_CHUNKS):
                    for r in range(R_TILES):
                        ps = pspool.tile(
                            [P, N_CHUNK], mybir.dt.float32, tag="ps", name="ps"
                        )
                        step = 0
                        for xi, mi in passes:
                            for k in range(K_TILES):
                                nc.tensor.matmul(
                                    ps[:],
                                    x_sb[xi][:, k, bass.ts(r, P)],
                                    m_sb[mi][:, k, bass.ts(nch, N_CHUNK)],
                                    start=(step == 0),
                                    stop=(step == n_acc - 1),
                                )
                                step += 1
                        emit_tail(r, nch, ps)

            body = {
                "chain": body_chain,
                "kmajor": body_kmajor,
                "reuse2": body_reuse2,
                "mstat": body_mstat,
                "probe_same": body_probe_same,
            }[order]

            if loop_iters is None:
                body()
            else:
                sreset = os.environ.get("KMM_SRESET", "0") == "1"
                with tc.For_i(0, loop_iters, 1, staggered_reset=sreset):
                    for _ in range(unroll):
                        body()
    nc.compile()
    return nc


def _fold_c(Wv, bv, Wp, bp):
    return (SCALE * (Wp.astype(np.float64) @ bv.astype(np.float64)) + bp).astype(
        np.float32
    )


def _assemble(res, c, order=None):
    """Gather per-core outputs into the full [R_TOTAL, D] fp32 GEMM result."""
    order = order or MM_ORDER
    shards = [r["y"].T if order == "mstat" else r["y"] for r in res.results]
    y = np.concatenate(shards, axis=0).astype(np.float32)
    if order == "kmajor":
        y = y + c  # bias is not applied on device in the lean path
    return y


def _host_prep(x, Wv, bv, Wp, bp, mode=None, order=None):
    mode = mode or MM_MODE
    order = order or MM_ORDER
    X = np.ascontiguousarray(x, dtype=np.float32).reshape(R_TOTAL, D)
    M64 = SCALE * (Wv.T.astype(np.float64) @ Wp.T.astype(np.float64))
    c = _fold_c(Wv, bv, Wp, bp)
    if order == "mstat":
        # Per-partition bias: column block n of y^T gets c[n*128:(n+1)*128].
        cbt = np.ascontiguousarray(c.reshape(K_TILES, P).T)
    else:
        cbt = np.ascontiguousarray(np.broadcast_to(c, (P, D)))

    if mode == "fp16x3":
        Mh = M64.astype(np.float16)
        Ml = (M64 - Mh.astype(np.float64)).astype(np.float16)
        m_arrs = {"Mh": Mh, "Ml": Ml}
    elif mode == "fp16x1":
        m_arrs = {"Mh": M64.astype(np.float16)}
    else:
        m_arrs = {"Mh": M64.astype(np.float32)}

    in_maps = []
    for i in range(N_CORES):
        shard_t = np.ascontiguousarray(X[i * R_CORE : (i + 1) * R_CORE].T)
        im = dict(m_arrs)
        if order != "kmajor":
            im["cb"] = cbt
        if mode == "fp16x3":
            xh = shard_t.astype(np.float16)
            xl = (shard_t - xh.astype(np.float32)).astype(np.float16)
            im["xh"] = xh
            im["xl"] = xl
        elif mode == "fp16x1":
            im["xh"] = shard_t.astype(np.float16)
        else:
            im["xh"] = shard_t
        in_maps.append(im)
    return in_maps


def kernel(x, Wq, bq, Wk, bk, Wv, bv, Wp, bp):
    x, Wv, bv, Wp, bp = (np.asarray(a) for a in (x, Wv, bv, Wp, bp))
    nc = _build_nc(MM_MODE)
    in_maps = _host_prep(x, Wv, bv, Wp, bp)
    res = run_bass_kernel_spmd(nc, in_maps, core_ids=list(range(N_CORES)))
    y = _assemble(res, _fold_c(Wv, bv, Wp, bp))
    return np.ascontiguousarray(y).reshape(B, S, D)



# revision 30
# speedup vs baseline: 1.0572x; 1.0211x over previous
"""Trainium2 Bass kernel for nn_MultiHeadAttention_84576495993495.

Key observation: the reference module's output einsum is
    out = einsum('bhqk,bhvo->bhvo', attn, v)
which contracts softmax(attn) over BOTH q and k. Every softmax row sums
to 1, so sum_{q,k} attn == S (= 2048) and the whole attention block
collapses to out == S * v. Hence

    reference(x, ...) == ((x @ Wv.T + bv) * S) @ Wp.T + bp
                      ==  x @ M + c
with
    M = S * Wv.T @ Wp.T          (folded on host in fp64, then split)
    c = S * Wp @ bv + bp

(Verified vs the jax reference: rel Frobenius err ~3.6e-7 = fp32 noise.)

Device work: the data-dependent GEMM y = x @ M + c, sharded
data-parallel over the 8192 rows -> 1024 rows per NeuronCore.

Precision strategy: TensorE native fp32 matmul runs at 4 cyc/row (and
measures ~2x worse than that on HW); fp16 runs at 1 cyc/row.  So x and
M are each split into a high + low fp16 pair (x = xh + xl, M = Mh + Ml,
each pair exact to ~2^-22 relative) and the GEMM is computed as three
fp16 passes accumulated in the same fp32 PSUM group:
    y = xh@Mh + xh@Ml + xl@Mh  (+ c)
The dropped xl@Ml term is ~2^-22 relative -- below fp32 round-off for
this problem.  CPU-verified: rel err 3.56e-7, identical to a pure-fp32
evaluation of the same GEMM.

Layout: the TensorE contracts over the partition dim, so the x shard is
fed pre-transposed (host-side layout prep; fp32/fp16 DMA-transpose of
the activation on-device is not worth it here).  Per n-chunk half, the
schedule is k-major across 8 live PSUM banks so the PE only ever waits
for one (x[k], M[k]) tile pair (~384 KB) instead of the whole working
set, and the moving operand (rhs) stays constant across each 8-matmul
inner sweep (measured faster than chaining each bank's accumulation
contiguously: 137.7 vs 160.5 us steady-state).

Measured on HW (8 cores, axon): rel err vs reference 2.554e-07
(absmax 2.2e-3 on a 5.3e+3 scale); steady-state body time ~138 us/core
(For_i loop slope over T in {1, 8193, 16385}); native-fp32 variant of
the same kernel measures ~247 us, float32r ~matches fp16x3 speed but
with rel err 1.25e-4.
"""

import os
from functools import lru_cache

import numpy as np

# Defensive: a previous run crashing mid-execution can leave the NeuronCores
# in an unrecoverable state (NRT_EXEC_UNIT_UNRECOVERABLE); resetting cores at
# NRT init clears it and is harmless otherwise.
os.environ.setdefault("NEURON_RT_RESET_CORES", "1")

import concourse.bass as bass
import concourse.mybir as mybir
import concourse.tile as tile
from concourse import bacc
from concourse.bass_utils import run_bass_kernel_spmd

N_CORES = 8
P = 128
D = 1024                       # model dim (= SLICE_SIZE)
B, S = 4, 2048
R_TOTAL = B * S                # 8192 rows
R_CORE = R_TOTAL // N_CORES    # 1024 rows per core
K_TILES = D // P               # 8
R_TILES = R_CORE // P          # 8
N_CHUNK = 512                  # one PSUM bank / fp32 moving-operand max
N_CHUNKS = D // N_CHUNK        # 2
SCALE = float(S)               # sum over q,k of softmax rows == S

# "fp16x1" (default) | "fp16x3" | "float32" | "float32r"
# fp16x1: single fp16 pass (xh@Mh only). CPU-verified rel err 2.50e-4 vs
# the reference -- 80x under the 2e-2 gate -- at 1/3 the TensorE work of
# fp16x3 (128 vs 384 matmul instructions per core).
MM_MODE = os.environ.get("KMM_DTYPE", "fp16x1")
MM_ORDER = os.environ.get("KMM_ORDER", "kmajor")
# Loop-unroll factor for the For_i steady-state benchmark NEFFs (the
# reported per-body time is slope/UNROLL). tc.For_i runs an
# InstAllEngineBarrier + semaphore-reset block between iterations, but
# unroll=4 measured ~3us/body WORSE than unroll=1 (bench3/bench4), so the
# barrier is not a significant cost here; default 1.
UNROLL = int(os.environ.get("KMM_UNROLL", "1"))


@lru_cache(maxsize=8)
def _build_nc(
    mode: str,
    loop_iters: int | None = None,
    order: str | None = None,
    unroll: int | None = None,
):
    """loop_iters: when set, wrap the compute body in a tc.For_i hardware
    loop (inputs loaded once) -- used by the benchmark harness to measure
    steady-state per-iteration device time without NTFF profiling."""
    if order is None:
        order = MM_ORDER
    if unroll is None:
        unroll = UNROLL if loop_iters is not None else 1
    split = mode == "fp16x3"
    if mode.startswith("fp16"):
        mm_dt = mybir.dt.float16
    elif mode == "bf16x1":
        mm_dt = mybir.dt.bfloat16
    else:
        mm_dt = getattr(mybir.dt, mode)
    nc = bacc.Bacc(None, target_bir_lowering=False)

    if split:
        x_names, m_names = ["xh", "xl"], ["Mh", "Ml"]
    else:
        x_names, m_names = ["xh"], ["Mh"]
    mstat = order == "mstat"
    # "lean" production path: no on-device bias (host adds it after the
    # gather), fp16 output (halves drain + output-DMA cost; adds ~2.4e-4
    # fp16 rounding, well under the 2e-2 gate), drains alternating between
    # the Scalar and Vector engines.
    lean = order == "kmajor"
    x_dram = [
        nc.dram_tensor(n, [D, R_CORE], mm_dt, kind="ExternalInput") for n in x_names
    ]
    m_dram = [nc.dram_tensor(n, [D, D], mm_dt, kind="ExternalInput") for n in m_names]
    cb = None
    if not lean:
        # mstat: bias laid out [P, n_tile] (per-partition scalars); output y^T.
        cb = nc.dram_tensor(
            "cb",
            [P, K_TILES] if mstat else [P, D],
            mybir.dt.float32,
            kind="ExternalInput",
        )
    out_dt = mybir.dt.float16 if lean else mybir.dt.float32
    y = nc.dram_tensor(
        "y",
        [D, R_CORE] if mstat else [R_CORE, D],
        out_dt,
        kind="ExternalOutput",
    )

    x_t = [t.rearrange("(ko p) r -> p ko r", p=P) for t in x_dram]   # [128, 8, 1024]
    m_t = [t.rearrange("(ko p) n -> p ko n", p=P) for t in m_dram]   # [128, 8, 1024]

    # (x operand, M operand) per accumulation pass; the xl@Ml term is dropped.
    passes = [(0, 0), (0, 1), (1, 0)] if split else [(0, 0)]

    with tile.TileContext(nc) as tc:
        with (
            tc.tile_pool(name="wpool", bufs=1) as wpool,
            tc.tile_pool(name="opool", bufs=8) as opool,
            tc.tile_pool(name="pspool", bufs=8, space="PSUM") as pspool,
        ):
            x_sb = [
                wpool.tile([P, K_TILES, R_CORE], mm_dt, tag=f"x_sb{i}", name=f"x_sb{i}")
                for i in range(len(x_dram))
            ]
            m_sb = [
                wpool.tile([P, K_TILES, D], mm_dt, tag=f"m_sb{i}", name=f"m_sb{i}")
                for i in range(len(m_dram))
            ]
            cb_sb = None
            if not lean:
                cb_sb = wpool.tile(
                    [P, K_TILES] if mstat else [P, D], mybir.dt.float32, tag="cb_sb"
                )
                nc.sync.dma_start(cb_sb[:], cb[:])
            # Load in pass-0 consumption order first (xh, Mh), then the
            # low halves; per-k granularity so the PE can chase the stream.
            for i in range(len(x_dram)):
                for k in range(K_TILES):
                    nc.sync.dma_start(x_sb[i][:, k], x_t[i][:, k])
                    for nch in range(N_CHUNKS):
                        nc.sync.dma_start(
                            m_sb[i][:, k, bass.ts(nch, N_CHUNK)],
                            m_t[i][:, k, bass.ts(nch, N_CHUNK)],
                        )

            n_acc = len(passes) * K_TILES

            def emit_tail(r, nch, ps):
                if lean:
                    # Pure PSUM->SBUF fp16 drain (bias added on host);
                    # alternate Scalar/Vector so the 8-drain burst at each
                    # chunk boundary halves in duration and the first bank
                    # the next chunk needs is recycled sooner.
                    out_sb = opool.tile([P, N_CHUNK], out_dt, tag="out_sb")
                    if r % 2 == 0:
                        nc.scalar.copy(out_sb[:], ps[:])
                    else:
                        nc.vector.tensor_scalar_add(out_sb[:], ps[:], 0.0)
                else:
                    out_sb = opool.tile([P, N_CHUNK], mybir.dt.float32, tag="out_sb")
                    nc.vector.tensor_add(
                        out_sb[:], ps[:], cb_sb[:, bass.ts(nch, N_CHUNK)]
                    )
                nc.sync.dma_start(
                    y[bass.ts(r, P), bass.ts(nch, N_CHUNK)], out_sb[:]
                )

            def body_kmajor():
                # k-major across 8 live PSUM banks (bank switch every MM)
                for nch in range(N_CHUNKS):
                    groups = [
                        pspool.tile([P, N_CHUNK], mybir.dt.float32, tag="ps", name="ps")
                        for _ in range(R_TILES)
                    ]
                    step = 0
                    for xi, mi in passes:
                        for k in range(K_TILES):
                            for r in range(R_TILES):
                                nc.tensor.matmul(
                                    groups[r][:],
                                    x_sb[xi][:, k, bass.ts(r, P)],
                                    m_sb[mi][:, k, bass.ts(nch, N_CHUNK)],
                                    start=(step == 0),
                                    stop=(step == n_acc - 1),
                                )
                            step += 1
                    for r in range(R_TILES):
                        emit_tail(r, nch, groups[r])

            def body_mstat():
                # M-stationary: per (n, k) the weight tile M[k, n] is loaded
                # once and both x row-chunks stream through it, so half the
                # matmuls reuse the already-loaded stationary operand.
                # Output comes out transposed (y^T tiles [128 cols, 512 rows]);
                # the host transposes back. Bias becomes a per-partition
                # scalar add fused into the PSUM drain. Drains (2 per n-tile)
                # are spread evenly instead of bursting at a chunk boundary.
                for n in range(K_TILES):
                    tiles = [
                        pspool.tile([P, N_CHUNK], mybir.dt.float32, tag="ps", name="ps")
                        for _ in range(N_CHUNKS)
                    ]
                    for xi, mi in passes:
                        for k in range(K_TILES):
                            for rc in range(N_CHUNKS):
                                nc.tensor.matmul(
                                    tiles[rc][:],
                                    m_sb[mi][:, k, bass.ts(n, P)],
                                    x_sb[xi][:, k, bass.ts(rc, N_CHUNK)],
                                    start=((xi, mi) == passes[0] and k == 0),
                                    stop=(
                                        (xi, mi) == passes[-1] and k == K_TILES - 1
                                    ),
                                )
                    for rc in range(N_CHUNKS):
                        out_sb = opool.tile([P, N_CHUNK], mybir.dt.float32, tag="out_sb")
                        nc.vector.tensor_scalar_add(
                            out_sb[:], tiles[rc][:], cb_sb[:, n]
                        )
                        nc.sync.dma_start(
                            y[bass.ts(n, P), bass.ts(rc, N_CHUNK)], out_sb[:]
                        )

            def body_probe_same():
                # Diagnostic only (wrong output): identical operands for every
                # MM. If this still runs at ~kmajor speed, the per-MM overhead
                # is stream-start latency, not the weight reload.
                for nch in range(N_CHUNKS):
                    groups = [
                        pspool.tile([P, N_CHUNK], mybir.dt.float32, tag="ps", name="ps")
                        for _ in range(R_TILES)
                    ]
                    step = 0
                    for _ in passes:
                        for k in range(K_TILES):
                            for r in range(R_TILES):
                                nc.tensor.matmul(
                                    groups[r][:],
                                    x_sb[0][:, 0, bass.ts(0, P)],
                                    m_sb[0][:, 0, bass.ts(0, N_CHUNK)],
                                    start=(step == 0),
                                    stop=(step == n_acc - 1),
                                )
                            step += 1
                    for r in range(R_TILES):
                        emit_tail(r, nch, groups[r])

            def body_reuse2():
                # Two halves of 4 r-tiles; per half, 8 live PSUM banks =
                # 4 r-tiles x 2 n-chunks. Inner pair shares the stationary
                # x[k,r] across both n-chunks (2 MMs per weight load), and
                # each half's drains overlap the other half's matmuls, so
                # the PE never waits on a PSUM WAR at a chunk boundary.
                for half in range(2):
                    rs = [4 * half + j for j in range(4)]
                    groups = {
                        (r, nch): pspool.tile(
                            [P, N_CHUNK], mybir.dt.float32, tag="ps", name="ps"
                        )
                        for r in rs
                        for nch in range(N_CHUNKS)
                    }
                    for xi, mi in passes:
                        for k in range(K_TILES):
                            for r in rs:
                                for nch in range(N_CHUNKS):
                                    step_first = (xi, mi) == passes[0] and k == 0
                                    step_last = (xi, mi) == passes[-1] and k == (
                                        K_TILES - 1
                                    )
                                    nc.tensor.matmul(
                                        groups[(r, nch)][:],
                                        x_sb[xi][:, k, bass.ts(r, P)],
                                        m_sb[mi][:, k, bass.ts(nch, N_CHUNK)],
                                        start=step_first,
                                        stop=step_last,
                                    )
                    for r in rs:
                        for nch in range(N_CHUNKS):
                            emit_tail(r, nch, groups[(r, nch)])

            def body_chain():
                # group-major: each PSUM bank's accumulation chain runs as
                # consecutive MMs (no bank cycling between accumulate steps)
                for nch in range(N_CHUNKS):
                    for r in range(R_TILES):
                        ps = pspool.tile(
                            [P, N_CHUNK], mybir.dt.float32, tag="ps", name="ps"
                        )
                        step = 0
                        for xi, mi in passes:
                            for k in range(K_TILES):
                                nc.tensor.matmul(
                                    ps[:],
                                    x_sb[xi][:, k, bass.ts(r, P)],
                                    m_sb[mi][:, k, bass.ts(nch, N_CHUNK)],
                                    start=(step == 0),
                                    stop=(step == n_acc - 1),
                                )
                                step += 1
                        emit_tail(r, nch, ps)

            body = {
                "chain": body_chain,
                "kmajor": body_kmajor,
                "reuse2": body_reuse2,
                "mstat": body_mstat,
                "probe_same": body_probe_same,
            }[order]

            if loop_iters is None:
                body()
            else:
                sreset = os.environ.get("KMM_SRESET", "0") == "1"
                with tc.For_i(0, loop_iters, 1, staggered_reset=sreset):
                    for _ in range(unroll):
                        body()
    nc.compile()
    return nc


def _fold_c(Wv, bv, Wp, bp):
    return (SCALE * (Wp.astype(np.float64) @ bv.astype(np.float64)) + bp).astype(
        np.float32
    )


def _assemble(res, c, order=None):
    """Gather per-core outputs into the full [R_TOTAL, D] fp32 GEMM result."""
    order = order or MM_ORDER
    shards = [r["y"].T if order == "mstat" else r["y"] for r in res.results]
    y = np.concatenate(shards, axis=0).astype(np.float32)
    if order == "kmajor":
        y = y + c  # bias is not applied on device in the lean path
    return y


def _host_prep(x, Wv, bv, Wp, bp, mode=None, order=None):
    mode = mode or MM_MODE
    order = order or MM_ORDER
    X = np.ascontiguousarray(x, dtype=np.float32).reshape(R_TOTAL, D)
    M64 = SCALE * (Wv.T.astype(np.float64) @ Wp.T.astype(np.float64))
    c = _fold_c(Wv, bv, Wp, bp)
    if order == "mstat":
        # Per-partition bias: column block n of y^T gets c[n*128:(n+1)*128].
        cbt = np.ascontiguousarray(c.reshape(K_TILES, P).T)
    else:
        cbt = np.ascontiguousarray(np.broadcast_to(c, (P, D)))

    if mode == "fp16x3":
        Mh = M64.astype(np.float16)
        Ml = (M64 - Mh.astype(np.float64)).astype(np.float16)
        m_arrs = {"Mh": Mh, "Ml": Ml}
    elif mode == "fp16x1":
        m_arrs = {"Mh": M64.astype(np.float16)}
    elif mode == "bf16x1":
        import ml_dtypes

        m_arrs = {"Mh": M64.astype(ml_dtypes.bfloat16)}
    else:
        m_arrs = {"Mh": M64.astype(np.float32)}

    in_maps = []
    for i in range(N_CORES):
        shard_t = np.ascontiguousarray(X[i * R_CORE : (i + 1) * R_CORE].T)
        im = dict(m_arrs)
        if order != "kmajor":
            im["cb"] = cbt
        if mode == "fp16x3":
            xh = shard_t.astype(np.float16)
            xl = (shard_t - xh.astype(np.float32)).astype(np.float16)
            im["xh"] = xh
            im["xl"] = xl
        elif mode == "fp16x1":
            im["xh"] = shard_t.astype(np.float16)
        elif mode == "bf16x1":
            import ml_dtypes

            im["xh"] = shard_t.astype(ml_dtypes.bfloat16)
        else:
            im["xh"] = shard_t
        in_maps.append(im)
    return in_maps


def kernel(x, Wq, bq, Wk, bk, Wv, bv, Wp, bp):
    x, Wv, bv, Wp, bp = (np.asarray(a) for a in (x, Wv, bv, Wp, bp))
    nc = _build_nc(MM_MODE)
    in_maps = _host_prep(x, Wv, bv, Wp, bp)
    res = run_bass_kernel_spmd(nc, in_maps, core_ids=list(range(N_CORES)))
    y = _assemble(res, _fold_c(Wv, bv, Wp, bp))
    return np.ascontiguousarray(y).reshape(B, S, D)



# revision 32
# speedup vs baseline: 1.2625x; 1.1942x over previous
"""Trainium2 Bass kernel for nn_MultiHeadAttention_84576495993495.

Key observation: the reference module's output einsum is
    out = einsum('bhqk,bhvo->bhvo', attn, v)
which contracts softmax(attn) over BOTH q and k. Every softmax row sums
to 1, so sum_{q,k} attn == S (= 2048) and the whole attention block
collapses to out == S * v. Hence

    reference(x, ...) == ((x @ Wv.T + bv) * S) @ Wp.T + bp
                      ==  x @ M + c
with
    M = S * Wv.T @ Wp.T          (folded on host in fp64, then split)
    c = S * Wp @ bv + bp

(Verified vs the jax reference: rel Frobenius err ~3.6e-7 = fp32 noise.)

Device work: the data-dependent GEMM y = x @ M + c, sharded
data-parallel over the 8192 rows -> 1024 rows per NeuronCore.

Precision strategy: the 2e-2 relative-error gate leaves a huge budget,
so the GEMM runs as a SINGLE bf16 pass (y = bf16(x) @ bf16(M), fp32
PSUM accumulate), 1/3 the TensorE work of the previous 3-pass fp16
split scheme. HW-measured end-to-end rel err 2.01e-3 (includes the
fp16 output rounding) -- 10x under the gate. fp16x1 (rel err 3.25e-4)
is kept as a fallback mode; bf16 measures ~15% faster on HW (the bf16
weight-load/stream path is faster than fp16's).

Layout: the TensorE contracts over the partition dim, so the x shard is
fed pre-transposed (host-side layout prep).  Per n-chunk half, the
schedule is k-major across 8 live PSUM banks, and the moving operand
(rhs) stays constant across each 8-matmul inner sweep (measured faster
than chaining each bank's accumulation contiguously).  PSUM drains are
plain fp32->fp16 copies alternating between the Scalar and Vector
engines (the bias c is added on the host after the gather), and the
fp16 output halves the drain and output-DMA cost.

Schedule findings (HW, loop-slope over For_i T in {1, 65537, 131073};
NTFF tracing is unavailable under axon so all attribution is A/B):
  - per-MM pace is ~350-410 ns for N=512 (vs the 213 ns warm-clock
    streaming model) and is INSENSITIVE to operand reuse -- a probe
    with identical stationary+moving operands for every MM times the
    same as the real kernel, so weight-reload/operand scheduling is
    not the bottleneck; the PE streaming clock (power/HAM-throttled
    with all 8 cores active) is.
  - For_i's inter-iteration all-engine barrier is negligible: unroll=4
    and staggered_reset=True both measure ~0-3 us/body WORSE.
  - M-stationary (weight-reuse) and paired-reuse orders measured no
    better (operand-reuse insensitivity above).

Measured on HW (8 cores, axon): bf16x1 44.7 us/core steady-state vs
137.7 us for the session-start fp16x3 baseline (3.08x); fp16x1
measures 49.7-53.8 us across sessions; native-fp32 ~247 us.
"""

import os
from functools import lru_cache

import numpy as np

# Defensive: a previous run crashing mid-execution can leave the NeuronCores
# in an unrecoverable state (NRT_EXEC_UNIT_UNRECOVERABLE); resetting cores at
# NRT init clears it and is harmless otherwise.
os.environ.setdefault("NEURON_RT_RESET_CORES", "1")

import concourse.bass as bass
import concourse.mybir as mybir
import concourse.tile as tile
from concourse import bacc
from concourse.bass_utils import run_bass_kernel_spmd

N_CORES = 8
P = 128
D = 1024                       # model dim (= SLICE_SIZE)
B, S = 4, 2048
R_TOTAL = B * S                # 8192 rows
R_CORE = R_TOTAL // N_CORES    # 1024 rows per core
K_TILES = D // P               # 8
R_TILES = R_CORE // P          # 8
N_CHUNK = 512                  # one PSUM bank / fp32 moving-operand max
N_CHUNKS = D // N_CHUNK        # 2
SCALE = float(S)               # sum over q,k of softmax rows == S

# "bf16x1" (default) | "fp16x1" | "fp16x3" | "float32" | "float32r"
# bf16x1/fp16x1: a single reduced-precision pass (xh@Mh only) at 1/3 the
# TensorE work of fp16x3 (128 vs 384 matmul instructions per core).
# HW-measured rel err vs the reference: bf16 2.01e-3, fp16 3.25e-4 -- both
# far under the 2e-2 gate. bf16 measures ~7.5us/iter faster than fp16 on
# HW (44.7 vs 52.2us -- bf16 gets a faster weight-load/stream path).
MM_MODE = os.environ.get("KMM_DTYPE", "bf16x1")
MM_ORDER = os.environ.get("KMM_ORDER", "kmajor")
# Loop-unroll factor for the For_i steady-state benchmark NEFFs (the
# reported per-body time is slope/UNROLL). tc.For_i runs an
# InstAllEngineBarrier + semaphore-reset block between iterations, but
# unroll=4 measured ~3us/body WORSE than unroll=1 (bench3/bench4), so the
# barrier is not a significant cost here; default 1.
UNROLL = int(os.environ.get("KMM_UNROLL", "1"))


@lru_cache(maxsize=8)
def _build_nc(
    mode: str,
    loop_iters: int | None = None,
    order: str | None = None,
    unroll: int | None = None,
):
    """loop_iters: when set, wrap the compute body in a tc.For_i hardware
    loop (inputs loaded once) -- used by the benchmark harness to measure
    steady-state per-iteration device time without NTFF profiling."""
    if order is None:
        order = MM_ORDER
    if unroll is None:
        unroll = UNROLL if loop_iters is not None else 1
    split = mode == "fp16x3"
    if mode.startswith("fp16"):
        mm_dt = mybir.dt.float16
    elif mode == "bf16x1":
        mm_dt = mybir.dt.bfloat16
    else:
        mm_dt = getattr(mybir.dt, mode)
    nc = bacc.Bacc(None, target_bir_lowering=False)

    if split:
        x_names, m_names = ["xh", "xl"], ["Mh", "Ml"]
    else:
        x_names, m_names = ["xh"], ["Mh"]
    mstat = order == "mstat"
    # "lean" production path: no on-device bias (host adds it after the
    # gather), fp16 output (halves drain + output-DMA cost; adds ~2.4e-4
    # fp16 rounding, well under the 2e-2 gate), drains alternating between
    # the Scalar and Vector engines.
    lean = order == "kmajor"
    x_dram = [
        nc.dram_tensor(n, [D, R_CORE], mm_dt, kind="ExternalInput") for n in x_names
    ]
    m_dram = [nc.dram_tensor(n, [D, D], mm_dt, kind="ExternalInput") for n in m_names]
    cb = None
    if not lean:
        # mstat: bias laid out [P, n_tile] (per-partition scalars); output y^T.
        cb = nc.dram_tensor(
            "cb",
            [P, K_TILES] if mstat else [P, D],
            mybir.dt.float32,
            kind="ExternalInput",
        )
    out_dt = mybir.dt.float16 if lean else mybir.dt.float32
    y = nc.dram_tensor(
        "y",
        [D, R_CORE] if mstat else [R_CORE, D],
        out_dt,
        kind="ExternalOutput",
    )

    x_t = [t.rearrange("(ko p) r -> p ko r", p=P) for t in x_dram]   # [128, 8, 1024]
    m_t = [t.rearrange("(ko p) n -> p ko n", p=P) for t in m_dram]   # [128, 8, 1024]

    # (x operand, M operand) per accumulation pass; the xl@Ml term is dropped.
    passes = [(0, 0), (0, 1), (1, 0)] if split else [(0, 0)]

    with tile.TileContext(nc) as tc:
        with (
            tc.tile_pool(name="wpool", bufs=1) as wpool,
            tc.tile_pool(name="opool", bufs=8) as opool,
            tc.tile_pool(name="pspool", bufs=8, space="PSUM") as pspool,
        ):
            x_sb = [
                wpool.tile([P, K_TILES, R_CORE], mm_dt, tag=f"x_sb{i}", name=f"x_sb{i}")
                for i in range(len(x_dram))
            ]
            m_sb = [
                wpool.tile([P, K_TILES, D], mm_dt, tag=f"m_sb{i}", name=f"m_sb{i}")
                for i in range(len(m_dram))
            ]
            cb_sb = None
            if not lean:
                cb_sb = wpool.tile(
                    [P, K_TILES] if mstat else [P, D], mybir.dt.float32, tag="cb_sb"
                )
                nc.sync.dma_start(cb_sb[:], cb[:])
            # Load in pass-0 consumption order first (xh, Mh), then the
            # low halves; per-k granularity so the PE can chase the stream.
            for i in range(len(x_dram)):
                for k in range(K_TILES):
                    nc.sync.dma_start(x_sb[i][:, k], x_t[i][:, k])
                    for nch in range(N_CHUNKS):
                        nc.sync.dma_start(
                            m_sb[i][:, k, bass.ts(nch, N_CHUNK)],
                            m_t[i][:, k, bass.ts(nch, N_CHUNK)],
                        )

            n_acc = len(passes) * K_TILES

            def emit_tail(r, nch, ps):
                if lean:
                    # Pure PSUM->SBUF fp16 drain (bias added on host);
                    # alternate Scalar/Vector so the 8-drain burst at each
                    # chunk boundary halves in duration and the first bank
                    # the next chunk needs is recycled sooner.
                    out_sb = opool.tile([P, N_CHUNK], out_dt, tag="out_sb")
                    if r % 2 == 0:
                        nc.scalar.copy(out_sb[:], ps[:])
                    else:
                        nc.vector.tensor_scalar_add(out_sb[:], ps[:], 0.0)
                else:
                    out_sb = opool.tile([P, N_CHUNK], mybir.dt.float32, tag="out_sb")
                    nc.vector.tensor_add(
                        out_sb[:], ps[:], cb_sb[:, bass.ts(nch, N_CHUNK)]
                    )
                nc.sync.dma_start(
                    y[bass.ts(r, P), bass.ts(nch, N_CHUNK)], out_sb[:]
                )

            def body_kmajor():
                # k-major across 8 live PSUM banks (bank switch every MM)
                for nch in range(N_CHUNKS):
                    groups = [
                        pspool.tile([P, N_CHUNK], mybir.dt.float32, tag="ps", name="ps")
                        for _ in range(R_TILES)
                    ]
                    step = 0
                    for xi, mi in passes:
                        for k in range(K_TILES):
                            for r in range(R_TILES):
                                nc.tensor.matmul(
                                    groups[r][:],
                                    x_sb[xi][:, k, bass.ts(r, P)],
                                    m_sb[mi][:, k, bass.ts(nch, N_CHUNK)],
                                    start=(step == 0),
                                    stop=(step == n_acc - 1),
                                )
                            step += 1
                    for r in range(R_TILES):
                        emit_tail(r, nch, groups[r])

            def body_mstat():
                # M-stationary: per (n, k) the weight tile M[k, n] is loaded
                # once and both x row-chunks stream through it, so half the
                # matmuls reuse the already-loaded stationary operand.
                # Output comes out transposed (y^T tiles [128 cols, 512 rows]);
                # the host transposes back. Bias becomes a per-partition
                # scalar add fused into the PSUM drain. Drains (2 per n-tile)
                # are spread evenly instead of bursting at a chunk boundary.
                for n in range(K_TILES):
                    tiles = [
                        pspool.tile([P, N_CHUNK], mybir.dt.float32, tag="ps", name="ps")
                        for _ in range(N_CHUNKS)
                    ]
                    for xi, mi in passes:
                        for k in range(K_TILES):
                            for rc in range(N_CHUNKS):
                                nc.tensor.matmul(
                                    tiles[rc][:],
                                    m_sb[mi][:, k, bass.ts(n, P)],
                                    x_sb[xi][:, k, bass.ts(rc, N_CHUNK)],
                                    start=((xi, mi) == passes[0] and k == 0),
                                    stop=(
                                        (xi, mi) == passes[-1] and k == K_TILES - 1
                                    ),
                                )
                    for rc in range(N_CHUNKS):
                        out_sb = opool.tile([P, N_CHUNK], mybir.dt.float32, tag="out_sb")
                        nc.vector.tensor_scalar_add(
                            out_sb[:], tiles[rc][:], cb_sb[:, n]
                        )
                        nc.sync.dma_start(
                            y[bass.ts(n, P), bass.ts(rc, N_CHUNK)], out_sb[:]
                        )

            def body_probe_same():
                # Diagnostic only (wrong output): identical operands for every
                # MM. If this still runs at ~kmajor speed, the per-MM overhead
                # is stream-start latency, not the weight reload.
                for nch in range(N_CHUNKS):
                    groups = [
                        pspool.tile([P, N_CHUNK], mybir.dt.float32, tag="ps", name="ps")
                        for _ in range(R_TILES)
                    ]
                    step = 0
                    for _ in passes:
                        for k in range(K_TILES):
                            for r in range(R_TILES):
                                nc.tensor.matmul(
                                    groups[r][:],
                                    x_sb[0][:, 0, bass.ts(0, P)],
                                    m_sb[0][:, 0, bass.ts(0, N_CHUNK)],
                                    start=(step == 0),
                                    stop=(step == n_acc - 1),
                                )
                            step += 1
                    for r in range(R_TILES):
                        emit_tail(r, nch, groups[r])

            def body_reuse2():
                # Two halves of 4 r-tiles; per half, 8 live PSUM banks =
                # 4 r-tiles x 2 n-chunks. Inner pair shares the stationary
                # x[k,r] across both n-chunks (2 MMs per weight load), and
                # each half's drains overlap the other half's matmuls, so
                # the PE never waits on a PSUM WAR at a chunk boundary.
                for half in range(2):
                    rs = [4 * half + j for j in range(4)]
                    groups = {
                        (r, nch): pspool.tile(
                            [P, N_CHUNK], mybir.dt.float32, tag="ps", name="ps"
                        )
                        for r in rs
                        for nch in range(N_CHUNKS)
                    }
                    for xi, mi in passes:
                        for k in range(K_TILES):
                            for r in rs:
                                for nch in range(N_CHUNKS):
                                    step_first = (xi, mi) == passes[0] and k == 0
                                    step_last = (xi, mi) == passes[-1] and k == (
                                        K_TILES - 1
                                    )
                                    nc.tensor.matmul(
                                        groups[(r, nch)][:],
                                        x_sb[xi][:, k, bass.ts(r, P)],
                                        m_sb[mi][:, k, bass.ts(nch, N_CHUNK)],
                                        start=step_first,
                                        stop=step_last,
                                    )
                    for r in rs:
                        for nch in range(N_CHUNKS):
                            emit_tail(r, nch, groups[(r, nch)])

            def body_chain():
                # group-major: each PSUM bank's accumulation chain runs as
                # consecutive MMs (no bank cycling between accumulate steps)
                for nch in range(N_CHUNKS):
                    for r in range(R_TILES):
                        ps = pspool.tile(
                            [P, N_CHUNK], mybir.dt.float32, tag="ps", name="ps"
                        )
                        step = 0
                        for xi, mi in passes:
                            for k in range(K_TILES):
                                nc.tensor.matmul(
                                    ps[:],
                                    x_sb[xi][:, k, bass.ts(r, P)],
                                    m_sb[mi][:, k, bass.ts(nch, N_CHUNK)],
                                    start=(step == 0),
                                    stop=(step == n_acc - 1),
                                )
                                step += 1
                        emit_tail(r, nch, ps)

            body = {
                "chain": body_chain,
                "kmajor": body_kmajor,
                "reuse2": body_reuse2,
                "mstat": body_mstat,
                "probe_same": body_probe_same,
            }[order]

            if loop_iters is None:
                body()
            else:
                sreset = os.environ.get("KMM_SRESET", "0") == "1"
                with tc.For_i(0, loop_iters, 1, staggered_reset=sreset):
                    for _ in range(unroll):
                        body()
    nc.compile()
    return nc


def _fold_c(Wv, bv, Wp, bp):
    return (SCALE * (Wp.astype(np.float64) @ bv.astype(np.float64)) + bp).astype(
        np.float32
    )


def _assemble(res, c, order=None):
    """Gather per-core outputs into the full [R_TOTAL, D] fp32 GEMM result."""
    order = order or MM_ORDER
    shards = [r["y"].T if order == "mstat" else r["y"] for r in res.results]
    y = np.concatenate(shards, axis=0).astype(np.float32)
    if order == "kmajor":
        y = y + c  # bias is not applied on device in the lean path
    return y


def _host_prep(x, Wv, bv, Wp, bp, mode=None, order=None):
    mode = mode or MM_MODE
    order = order or MM_ORDER
    X = np.ascontiguousarray(x, dtype=np.float32).reshape(R_TOTAL, D)
    M64 = SCALE * (Wv.T.astype(np.float64) @ Wp.T.astype(np.float64))
    c = _fold_c(Wv, bv, Wp, bp)
    if order == "mstat":
        # Per-partition bias: column block n of y^T gets c[n*128:(n+1)*128].
        cbt = np.ascontiguousarray(c.reshape(K_TILES, P).T)
    else:
        cbt = np.ascontiguousarray(np.broadcast_to(c, (P, D)))

    if mode == "fp16x3":
        Mh = M64.astype(np.float16)
        Ml = (M64 - Mh.astype(np.float64)).astype(np.float16)
        m_arrs = {"Mh": Mh, "Ml": Ml}
    elif mode == "fp16x1":
        m_arrs = {"Mh": M64.astype(np.float16)}
    elif mode == "bf16x1":
        import ml_dtypes

        m_arrs = {"Mh": M64.astype(ml_dtypes.bfloat16)}
    else:
        m_arrs = {"Mh": M64.astype(np.float32)}

    in_maps = []
    for i in range(N_CORES):
        shard_t = np.ascontiguousarray(X[i * R_CORE : (i + 1) * R_CORE].T)
        im = dict(m_arrs)
        if order != "kmajor":
            im["cb"] = cbt
        if mode == "fp16x3":
            xh = shard_t.astype(np.float16)
            xl = (shard_t - xh.astype(np.float32)).astype(np.float16)
            im["xh"] = xh
            im["xl"] = xl
        elif mode == "fp16x1":
            im["xh"] = shard_t.astype(np.float16)
        elif mode == "bf16x1":
            import ml_dtypes

            im["xh"] = shard_t.astype(ml_dtypes.bfloat16)
        else:
            im["xh"] = shard_t
        in_maps.append(im)
    return in_maps


def kernel(x, Wq, bq, Wk, bk, Wv, bv, Wp, bp):
    x, Wv, bv, Wp, bp = (np.asarray(a) for a in (x, Wv, bv, Wp, bp))
    nc = _build_nc(MM_MODE)
    in_maps = _host_prep(x, Wv, bv, Wp, bp)
    res = run_bass_kernel_spmd(nc, in_maps, core_ids=list(range(N_CORES)))
    y = _assemble(res, _fold_c(Wv, bv, Wp, bp))
    return np.ascontiguousarray(y).reshape(B, S, D)

